# revision 2
# baseline (speedup 1.0000x reference)
"""Trainium2 Bass kernel for nn_EqvRESFeedForward — v2.

Strategy (vs baseline): refit r -> R(r) onto D=14 equally-spaced Gaussians
(s=0.88, BW=1.10), evaluated on device via a multiplicative chain:
  u_k = anchor * q^(k-ANCH),  q = exp(4 s r / BW^2)
with all d-dependent constants folded into the fitted coefficients Q'.
Per rep this costs ~8 ACT ops + 6 DVE/Pool mults instead of per-d
Square+Exp pairs.  Partitions carry (b, m'); parity lives in the free dim,
so r-geometry ops are not parity-duplicated.  Conv matmuls pack both
batches into one stationary [z_b0 | z_b1] (off-batch halves zeroed), f32r
everywhere (no bf16 error-compensation matmuls).  The final AllReduce is
piggybacked onto the next rep's ReduceScatter (8 replicated 128B slots),
so steady state runs ONE collective per rep.
"""
import os
import sys
import time

import numpy as np

for _p in ("/opt/trn_rl_repo", "/root/.axon_site/_ro/trn_rl_repo"):
    if os.path.isdir(_p) and _p not in sys.path:
        sys.path.insert(0, _p)

import concourse.bacc as bacc
import concourse.bass as bass
import concourse.mybir as mybir
import concourse.tile as tile
from concourse.bass_utils import run_bass_kernel_spmd

# ---- problem constants (hardcoded per contract) ----
B, N, C = 2, 384, 16
NB, H = 10, 64
MAX_RADIUS = 10.0
WIDTH = MAX_RADIUS / NB
N_CORES = 8
MS = N // N_CORES          # m-slice per core = 48
MP = 64                    # m padded to 64; partitions = (b, m') = 128
EPS_R2 = 1e-3

# ---- basis-fit hyperparameters (validated e2e ~2.1e-3 in numpy sim) ----
D = 14
DH = D // 2                # 7 dhi values; d = 2*dhi + par
S = 0.88                   # center spacing
BW = 1.10                  # basis width
C0 = 0.0                   # first center
ANCH = 3                   # anchor dhi (mid: avoids fp32 over/underflow)
FIT_RMAX = MAX_RADIUS * np.sqrt(3.0) + 0.1
FIT_GRID = 4096
FIT_LAM = 1e-9

AF = mybir.ActivationFunctionType
ALU = mybir.AluOpType
AX = mybir.AxisListType
F32 = mybir.dt.float32
F32R = mybir.dt.float32r
BF16 = mybir.dt.bfloat16

_CACHE = {}


# ----------------------------------------------------------------------
# host-side prep (numpy; only O(N)/O(weights) work — no pairwise compute)
# ----------------------------------------------------------------------

def _fit_weight():
    """sqrt(pair-distance density) for U[0,10]^3, floored; cached."""
    if "w" not in _CACHE:
        rng = np.random.default_rng(1)
        a = rng.uniform(0, MAX_RADIUS, (100000, 3))
        b = rng.uniform(0, MAX_RADIUS, (100000, 3))
        dd = np.linalg.norm(a - b, axis=1)
        hist, edges = np.histogram(dd, bins=128, range=(0, FIT_RMAX),
                                   density=True)
        _CACHE["w"] = (hist, 0.5 * (edges[1:] + edges[:-1]))
    hist, cent = _CACHE["w"]
    rg = np.linspace(0.0, FIT_RMAX, FIT_GRID)
    return rg, np.sqrt(np.interp(rg, cent, hist) + 0.02)


def _fit_q(w1, w2):
    """Weighted least-squares refit of r -> swish(rbf(r)@w1)@w2 onto the
    D Gaussian basis.  Returns Q [D, C*C] float64."""
    cen = C0 + S * np.arange(D)
    rg, w = _fit_weight()
    phi = np.exp(-(((rg[:, None] - cen) / BW) ** 2))
    rbf = np.exp(-(((rg[:, None] - np.linspace(0.0, MAX_RADIUS, NB))
                    / WIDTH) ** 2))
    pre = rbf @ w1.astype(np.float64)
    hid = pre / (1.0 + np.exp(-pre))
    tgt = hid @ w2.astype(np.float64)
    pw = phi * w[:, None]
    a = pw.T @ phi + FIT_LAM * np.eye(D)
    return np.linalg.solve(a, pw.T @ tgt)        # [D, C*C]


def _fold_g():
    """g_d such that phi_d = u_{dhi,par} * g_d (fold into Q)."""
    g = np.zeros(D)
    for dd in range(D):
        k, p = dd // 2, dd % 2
        dk = k - ANCH
        a_p = (2 * ANCH + p) * S + C0
        g[dd] = np.exp(-4 * dk * S * a_p / BW**2 - (2 * dk * S / BW) ** 2)
    return g


def _qeo(q, g, scale):
    """rhs of the z matmul: [C, 2*DH*C]; qeo[j, (par, dhi, i)] =
    Q'_d[i, j], d = 2*dhi+par, Q' = Q * g * scale."""
    qp = (q * (g * scale)[:, None]).astype(np.float32)    # [D, C*C]
    qr = qp.reshape(D, C, C)                              # [d, i, j]
    out = np.zeros((C, 2, DH, C), np.float32)
    for dd in range(D):
        k, p = dd // 2, dd % 2
        out[:, p, k, :] = qr[dd].T                        # [j, i]
    return out.reshape(C, 2 * DH * C)


def _actc():
    """Per-partition constant columns for ACT scale/bias APs."""
    c = np.zeros((128, 8), np.float32)
    c[:, 0] = 0.5
    c[:, 1] = -1.0
    c[:, 2] = -np.log(BW)
    c[:, 3] = 4.0 * S / BW                  # alpha: q = exp(alpha*rs)
    c[:, 4] = -(2 * ANCH * S + C0) / BW     # anchor bias par 0
    c[:, 5] = -((2 * ANCH + 1) * S + C0) / BW  # anchor bias par 1
    c[:, 6] = -4.0 * S / BW                 # -alpha for qi
    c[:, 7] = 1.0 / (C - 1)                 # tail: unbiased-var scale
    return c


def _host_prep(x, xyz, mask, conv1_w1, conv1_w2, conv2_w1, conv2_w2, fc2_w):
    x = np.asarray(x, np.float32)
    xyz = np.asarray(xyz, np.float32)
    mask = np.asarray(mask)
    diag = np.einsum('bnn->bn', mask)
    keep = (diag != 0).astype(np.float32)                 # [B, N]

    g = _fold_g()
    inv_sqrt_n = 1.0 / np.sqrt(np.float64(N))
    q1 = _fit_q(np.asarray(conv1_w1), np.asarray(conv1_w2))
    q2 = _fit_q(np.asarray(conv2_w1), np.asarray(conv2_w2))
    qeo = np.stack([_qeo(q1, g, inv_sqrt_n), _qeo(q2, g, inv_sqrt_n)])

    # keep32[(b,i), n] = keep[b, n]  (ttr mask for the conv2 node-sum)
    keep32 = np.broadcast_to(keep[:, None, :], (B, C, N)).reshape(
        B * C, N).astype(np.float32).copy()

    # geo rhs, K=10: rows 5b..5b+4 = [xn; |xn|^2; 1] of batch b
    grh = np.zeros((2 * 5, N), np.float32)
    for b in range(B):
        grh[5 * b:5 * b + 3, :] = xyz[b].T
        grh[5 * b + 3, :] = np.sum(xyz[b] * xyz[b], axis=1)
        grh[5 * b + 4, :] = 1.0

    fc2t = np.ascontiguousarray(np.asarray(fc2_w, np.float32).T)
    ones16 = np.ones((C, 1), np.float32)
    ident2 = np.eye(2, dtype=np.float32)
    ident16 = np.eye(C, dtype=np.float32)
    ident32 = np.eye(2 * C, dtype=np.float32)

    xk = x * keep[:, :, None]                             # masked conv1 input

    in_maps = []
    for c in range(N_CORES):
        sl = slice(c * MS, (c + 1) * MS)
        xm = xyz[:, sl, :]                                # [B, 48, 3]
        # glh[5b:5b+5, b*64+m'] = [-2x, -2y, -2z, 1, |xm|^2+eps]; off-b 0
        glh = np.zeros((2 * 5, 128), np.float32)
        for b in range(B):
            col = np.zeros((5, MP), np.float32)
            col[0:3, :MS] = -2.0 * xm[b].T
            col[3, :] = 1.0
            col[4, :MS] = np.sum(xm[b] * xm[b], axis=1) + EPS_R2
            col[4, MS:] = 1.0
            glh[5 * b:5 * (b + 1), b * MP:(b + 1) * MP] = col
        # x0t_b[j, (b',m')] = keep*x transposed; other-b half zeroed
        x0t = np.zeros((B, C, 2 * MP), np.float32)
        for b in range(B):
            x0t[b, :, b * MP:b * MP + MS] = np.transpose(xk[b, sl, :])
        in_maps.append(dict(
            glh=glh, grh=grh, x0t=x0t, qeo=qeo, keep32=keep32,
            fc2T=fc2t, ones16=ones16, ident2=ident2, ident16=ident16,
            ident32=ident32, actc=_actc(),
        ))
    return in_maps


# ----------------------------------------------------------------------
# device program
# ----------------------------------------------------------------------

PAY = B * C * MS            # 1536: x1 payload floats per RS slot
PAYS = PAY + 2 * C          # + 32 piggybacked partial-sum floats


def _build_nc(reps=1, pig=True, nocoll=False):
    nc = bacc.Bacc("TRN2", target_bir_lowering=False, debug=False,
                   num_devices=N_CORES)
    d_glh = nc.dram_tensor("glh", [10, 128], F32, kind="ExternalInput")
    d_grh = nc.dram_tensor("grh", [10, N], F32, kind="ExternalInput")
    d_x0 = nc.dram_tensor("x0t", [B, C, 2 * MP], F32, kind="ExternalInput")
    d_qeo = nc.dram_tensor("qeo", [2, C, 2 * DH * C], F32,
                           kind="ExternalInput")
    d_keep = nc.dram_tensor("keep32", [B * C, N], F32, kind="ExternalInput")
    d_fc2t = nc.dram_tensor("fc2T", [C, C], F32, kind="ExternalInput")
    d_ones = nc.dram_tensor("ones16", [C, 1], F32, kind="ExternalInput")
    d_id2 = nc.dram_tensor("ident2", [2, 2], F32, kind="ExternalInput")
    d_id16 = nc.dram_tensor("ident16", [C, C], F32, kind="ExternalInput")
    d_id32 = nc.dram_tensor("ident32", [2 * C, 2 * C], F32,
                            kind="ExternalInput")
    d_actc = nc.dram_tensor("actc", [128, 8], F32, kind="ExternalInput")
    d_out = nc.dram_tensor("out", [B, C], F32, kind="ExternalOutput")

    groups = [list(range(N_CORES))]

    def f32r(ap):
        return ap.bitcast(F32R)

    with tile.TileContext(nc) as tc:
        with (
            tc.tile_pool(name="const", bufs=1) as cpool,
            tc.tile_pool(name="basis", bufs=2) as bpool,
            tc.tile_pool(name="work", bufs=2) as wpool,
            tc.tile_pool(name="psr", bufs=2, space="PSUM") as psr,
            tc.tile_pool(name="psz", bufs=1, space="PSUM") as psz,
            tc.tile_pool(name="psc", bufs=2, space="PSUM") as psc,
            tc.tile_pool(name="pst", bufs=1, space="PSUM") as pst,
            tc.tile_pool(name="dram", bufs=2, space="DRAM") as dram,
        ):
            # ---- constants: loaded once, reused by every rep ----
            glh_sb = cpool.tile([10, 128], F32, tag="glh")
            nc.sync.dma_start(out=glh_sb[:], in_=d_glh[:])
            grh_sb = cpool.tile([10, N], F32, tag="grh")
            nc.sync.dma_start(out=grh_sb[:], in_=d_grh[:])
            x0t_sb = cpool.tile([C, B, 2 * MP], F32, tag="x0t")
            nc.sync.dma_start(out=x0t_sb[:],
                              in_=d_x0[:].rearrange("b j m -> j b m"))
            qeo_sb = cpool.tile([C, 2, 2 * DH * C], F32, tag="qeo")
            nc.sync.dma_start(out=qeo_sb[:],
                              in_=d_qeo[:].rearrange("l j e -> j l e"))
            keep_sb = cpool.tile([2 * C, N], F32, tag="keep")
            nc.sync.dma_start(out=keep_sb[:], in_=d_keep[:])
            fc2t_sb = cpool.tile([C, C], F32, tag="fc2t")
            nc.sync.dma_start(out=fc2t_sb[:], in_=d_fc2t[:])
            ones_sb = cpool.tile([C, 1], F32, tag="ones")
            nc.sync.dma_start(out=ones_sb[:], in_=d_ones[:])
            id2_sb = cpool.tile([2, 2], F32, tag="id2")
            nc.sync.dma_start(out=id2_sb[:], in_=d_id2[:])
            id16_sb = cpool.tile([C, C], F32, tag="id16")
            nc.sync.dma_start(out=id16_sb[:], in_=d_id16[:])
            id32_sb = cpool.tile([2 * C, 2 * C], F32, tag="id32")
            nc.sync.dma_start(out=id32_sb[:], in_=d_id32[:])
            x0t_r = cpool.tile([C, B, 2 * MP], F32R, tag="x0tr")
            nc.vector.tensor_copy(x0t_r[:], x0t_sb[:])
            qeo_r = cpool.tile([C, 2, 2 * DH * C], F32R, tag="qeor")
            nc.vector.tensor_copy(qeo_r[:], qeo_sb[:])
            actc = cpool.tile([128, 8], F32, tag="actc")
            nc.sync.dma_start(out=actc[:], in_=d_actc[:])
            c_half = actc[:, 0:1]
            c_neg1 = actc[:, 1:2]
            c_lnbw = actc[:, 2:3]
            c_alpha = actc[:, 3:4]
            c_anch0 = actc[:, 4:5]
            c_anch1 = actc[:, 5:6]
            c_nalpha = actc[:, 6:7]
            c_i15 = actc[0:B, 7:8]           # 1/(C-1) for tail var scale

            prev_s = None        # piggyback state: rep k-1's tail pending

            def tail(s2d_src, dest):
                """normalize(ddof=1) + fc2 + softmax on s [B, C] -> dest."""
                s2db = wpool.tile([B, C], BF16, tag="s2db")
                nc.sync.dma_start(out=s2db[:], in_=s2d_src)
                s2d = wpool.tile([B, C], F32, tag="s2d")
                nc.vector.tensor_copy(s2d[:], s2db[:])
                musum = wpool.tile([B, 1], F32, tag="musum")
                nc.vector.reduce_sum(musum[:], s2d[:], axis=AX.X)
                mu = wpool.tile([B, 1], F32, tag="mu")
                nc.vector.tensor_scalar_mul(mu[:], musum[:], 1.0 / C)
                cen = wpool.tile([B, C], F32, tag="cen")
                nc.vector.tensor_scalar(out=cen[:], in0=s2d[:],
                                        scalar1=mu[:], scalar2=None,
                                        op0=ALU.subtract)
                sq2 = wpool.tile([B, C], F32, tag="sq2")
                nc.vector.tensor_tensor(out=sq2[:], in0=cen[:], in1=cen[:],
                                        op=ALU.mult)
                varsum = wpool.tile([B, 1], F32, tag="varsum")
                nc.vector.reduce_sum(varsum[:], sq2[:], axis=AX.X)
                lnv = wpool.tile([B, 1], F32, tag="lnv")
                nc.scalar.activation(lnv[:], varsum[:], AF.Ln, scale=c_i15)
                std = wpool.tile([B, 1], F32, tag="std")
                nc.scalar.activation(std[:], lnv[:], AF.Exp,
                                     scale=c_half[0:B])
                stde = wpool.tile([B, 1], F32, tag="stde")
                nc.vector.tensor_scalar_add(stde[:], std[:], 1e-6)
                rinv = wpool.tile([B, 1], F32, tag="rinv")
                nc.vector.reciprocal(rinv[:], stde[:])
                normed = wpool.tile([B, C], F32, tag="normed")
                nc.vector.tensor_scalar_mul(normed[:], cen[:], rinv[:])
                ps_nt = pst.tile([C, B], F32, tag="tail")
                nc.tensor.transpose(ps_nt[:], normed[:], id2_sb[:])
                nt = wpool.tile([C, B], F32, tag="nt")
                nc.vector.tensor_copy(nt[:], ps_nt[:])
                ps_l = pst.tile([C, B], F32, tag="tail")
                nc.tensor.matmul(ps_l[:], fc2t_sb[:], nt[:],
                                 start=True, stop=True)
                el = wpool.tile([C, B], F32, tag="el")
                nc.scalar.activation(el[:], ps_l[:], AF.Exp)
                ps_den = pst.tile([B, 1], F32, tag="tail")
                nc.tensor.matmul(ps_den[:], el[:], ones_sb[:],
                                 start=True, stop=True)
                den = wpool.tile([B, 1], F32, tag="den")
                nc.vector.tensor_copy(den[:], ps_den[:])
                rden = wpool.tile([B, 1], F32, tag="rden")
                nc.vector.reciprocal(rden[:], den[:])
                ps_e2 = pst.tile([B, C], F32, tag="tail")
                nc.tensor.transpose(ps_e2[:], el[:], id16_sb[:])
                outf = wpool.tile([B, C], F32, tag="outf")
                nc.vector.tensor_scalar_mul(outf[:], ps_e2[:], rden[:])
                nc.sync.dma_start(out=dest, in_=outf[:])

            for _rep in range(reps):
                # --- r^2 for both b: partitions (b, m'), free n ---
                ps_r2 = psr.tile([128, N], F32, tag="r2")
                nc.tensor.matmul(ps_r2[:], glh_sb[:], grh_sb[:],
                                 start=True, stop=True)
                lnr2 = wpool.tile([128, N], F32, tag="lnr2")
                nc.scalar.activation(lnr2[:], ps_r2[:], AF.Ln)
                rs = bpool.tile([128, N], F32, tag="rs")
                nc.scalar.activation(rs[:], lnr2[:], AF.Exp,
                                     scale=c_half, bias=c_lnbw)
                q = bpool.tile([128, N], F32, tag="q")
                nc.scalar.activation(q[:], rs[:], AF.Exp, scale=c_alpha)
                qi = bpool.tile([128, N], F32, tag="qi")
                nc.scalar.activation(qi[:], rs[:], AF.Exp, scale=c_nalpha)

                # u tiles [128, 2par, N]; u3 = anchors via Square+Exp
                u = [bpool.tile([128, 2, N], F32R, tag=f"u{k}",
                                name=f"u{k}") for k in range(DH)]
                sqa = wpool.tile([128, 2, N], F32, tag="sqa")
                nc.scalar.activation(sqa[:, 0, :], rs[:], AF.Square,
                                     bias=c_anch0)
                nc.scalar.activation(sqa[:, 1, :], rs[:], AF.Square,
                                     bias=c_anch1)
                nc.scalar.activation(u[ANCH][:], sqa[:], AF.Exp,
                                     scale=c_neg1)

                def bmul(eng, dst, a, mul):
                    eng.tensor_tensor(
                        out=dst[:], in0=a[:].bitcast(F32),
                        in1=mul[:].unsqueeze(1).broadcast_to((128, 2, N)),
                        op=ALU.mult)

                bmul(nc.vector, u[ANCH + 1], u[ANCH], q)    # u4
                bmul(nc.vector, u[ANCH + 2], u[ANCH + 1], q)  # u5
                bmul(nc.vector, u[ANCH + 3], u[ANCH + 2], q)  # u6
                bmul(nc.gpsimd, u[ANCH - 1], u[ANCH], qi)   # u2
                bmul(nc.gpsimd, u[ANCH - 2], u[ANCH - 1], qi)  # u1
                bmul(nc.vector, u[ANCH - 3], u[ANCH - 2], qi)  # u0

                rs_in = dram.tile([N_CORES, PAYS], BF16, tag="rsin",
                                  name="rsin")
                rs_out = dram.tile([PAYS], BF16, tag="rsout", name="rsout")

                def zmm(xta, xtb, lay):
                    """z for both b: two ap-224 matmuls -> zw [128,2,7,32].
                    zw free layout (par, dhi, bhat*C+i); xta/xtb are [C, 128]
                    with the off-batch partition half zeroed."""
                    pza = psz.tile([128, 2 * DH * C], F32, tag="za")
                    pzb = psz.tile([128, 2 * DH * C], F32, tag="zb")
                    nc.tensor.matmul(pza[:], xta, qeo_r[:, lay, :],
                                     start=True, stop=True)
                    nc.tensor.matmul(pzb[:], xtb, qeo_r[:, lay, :],
                                     start=True, stop=True)
                    zw = wpool.tile([128, 2, DH, 2 * C], F32R, tag="zw")
                    nc.vector.tensor_copy(
                        zw[:, :, :, 0:C],
                        pza[:].rearrange("p (a k i) -> p a k i", a=2, k=DH))
                    nc.vector.tensor_copy(
                        zw[:, :, :, C:2 * C],
                        pzb[:].rearrange("p (a k i) -> p a k i", a=2, k=DH))
                    return zw

                def conv(zw):
                    ps_c = psc.tile([2 * C, N], F32, tag="c")
                    t = 0
                    for k in range(DH):
                        for par in range(2):
                            nc.tensor.matmul(
                                ps_c[:], zw[:, par, k, :],
                                u[k][:, par, :],
                                start=(t == 0), stop=(t == 2 * DH - 1))
                            t += 1
                    return ps_c

                # --- conv1 ---
                zw1 = zmm(x0t_r[:, 0, :], x0t_r[:, 1, :], 0)
                ps_c1 = conv(zw1)
                x1p = wpool.tile([2 * C, N], BF16, tag="x1p")
                nc.scalar.activation(x1p[:], ps_c1[:], AF.Copy)
                nc.sync.dma_start(
                    out=rs_in[:, 0:PAY].rearrange(
                        "c (p m) -> p c m", p=B * C),
                    in_=x1p[:].rearrange("p (c m) -> p c m", c=N_CORES))

                # piggyback previous rep's partial sums (slot-replicated)
                if pig:
                    if prev_s is None:
                        prev_s = wpool.tile([1, 2 * C], BF16, tag="ssb")
                        nc.vector.memset(prev_s[:], 0.0)
                        first = True
                    else:
                        first = False
                    s8 = wpool.tile([1, N_CORES, 2 * C], BF16, tag="s8")
                    nc.vector.tensor_copy(
                        s8[:], prev_s[:].unsqueeze(1)
                        .broadcast_to((1, N_CORES, 2 * C)))
                    nc.sync.dma_start(out=rs_in[:, PAY:PAYS], in_=s8[0:1])

                if nocoll:
                    nc.sync.dma_start(out=rs_out[:], in_=rs_in[0])
                else:
                    nc.gpsimd.collective_compute(
                        "ReduceScatter", ALU.add, replica_groups=groups,
                        ins=[rs_in.opt()], outs=[rs_out.opt()])

                if pig and not first:
                    osc = dram.tile([B, C], F32, tag="oscratch",
                                    name="oscratch")
                    tail(rs_out[PAY:PAYS].rearrange("(b i) -> b i", b=B),
                         osc[:])

                # --- conv2 on the scattered x1 slice ---
                x1raw = wpool.tile([C, B, MS], BF16, tag="x1raw")
                nc.sync.dma_start(
                    out=x1raw[:],
                    in_=rs_out[0:PAY].rearrange("(b i m) -> i b m", b=B, i=C))
                x1t = [wpool.tile([C, 2 * MP], F32R, tag=f"x1t{b}",
                                  name=f"x1t{b}") for b in range(B)]
                for b in range(B):
                    nc.gpsimd.memset(x1t[b][:].bitcast(F32), 0.0)
                    nc.gpsimd.tensor_copy(x1t[b][:, b * MP:b * MP + MS],
                                          x1raw[:, b, :])
                zw2 = zmm(x1t[0][:], x1t[1][:], 1)
                ps_c2 = conv(zw2)

                # masked node-sum: one fused multiply+reduce over [32, N]
                xm2 = wpool.tile([2 * C, N], F32, tag="xm2")
                s32 = wpool.tile([2 * C, 1], F32, tag="s32")
                nc.vector.tensor_tensor(out=xm2[:], in0=ps_c2[:],
                                        in1=keep_sb[:], op=ALU.mult)
                nc.vector.reduce_sum(s32[:], xm2[:], axis=AX.X)
                # transpose partials to [1, 32] for cheap replication DMA
                ps_s = pst.tile([1, 2 * C], F32, tag="pss")
                nc.tensor.transpose(ps_s[:], s32[:], id32_sb[:])
                s_sb = wpool.tile([1, 2 * C], BF16, tag="ssb")
                nc.vector.tensor_copy(s_sb[:], ps_s[:])
                prev_s = s_sb
                if not pig:
                    ar_in = dram.tile([1, 2 * C], BF16, tag="arin",
                                      name="arin")
                    ar_out = dram.tile([1, 2 * C], BF16, tag="arout",
                                       name="arout")
                    nc.sync.dma_start(out=ar_in[:], in_=prev_s[:])
                    if nocoll:
                        nc.sync.dma_start(out=ar_out[:], in_=ar_in[:])
                    else:
                        nc.gpsimd.collective_compute(
                            "AllReduce", ALU.add, replica_groups=groups,
                            ins=[ar_in.opt()], outs=[ar_out.opt()])
                    dest = d_out[:] if _rep == reps - 1 else dram.tile(
                        [B, C], F32, tag="oscratch", name="oscratch")[:]
                    tail(ar_out[:], dest)

            if pig:
                # trailing: reduce last rep's partials, final tail -> out
                ar_in = dram.tile([1, 2 * C], BF16, tag="arin", name="arin")
                ar_out = dram.tile([1, 2 * C], BF16, tag="arout", name="arout")
                nc.sync.dma_start(out=ar_in[:], in_=prev_s[:])
                if nocoll:
                    nc.sync.dma_start(out=ar_out[:], in_=ar_in[:])
                else:
                    nc.gpsimd.collective_compute(
                        "AllReduce", ALU.add, replica_groups=groups,
                        ins=[ar_in.opt()], outs=[ar_out.opt()])
                tail(ar_out[:], d_out[:])

    nc.compile()
    return nc


def get_nc(reps=1, pig=True, nocoll=False):
    key = ("nc", reps, pig, nocoll)
    if key not in _CACHE:
        _CACHE[key] = _build_nc(reps, pig, nocoll)
    return _CACHE[key]


def kernel(x, xyz, mask, conv1_w1, conv1_w2, conv2_w1, conv2_w2, fc2_w,
           _return_results=False, **_unused):
    nc = get_nc()
    in_maps = _host_prep(x, xyz, mask, conv1_w1, conv1_w2,
                         conv2_w1, conv2_w2, fc2_w)
    res = None
    last_err = None
    for attempt in range(4):
        try:
            res = run_bass_kernel_spmd(nc, in_maps,
                                       core_ids=list(range(N_CORES)))
            break
        except Exception as e:  # transient NRT/axon wedges recover in ~10-30s
            last_err = e
            time.sleep(10.0 * (attempt + 1))
    if res is None:
        raise last_err
    if _return_results:
        return res
    return np.asarray(res.results[0]["out"], np.float32)
